# revision 22
# baseline (speedup 1.0000x reference)
"""Trainium2 Bass kernel for BaseCausalWanSelfAttention (local+sink sparse attention
with interleaved rotary), SPMD across 8 NeuronCores.

Sharding: the 24 (batch, head) pairs are split 3-per-core across 8 cores; each
core runs full local+sink attention for its pairs independently (no collectives).

v6: host-side rotary+layouts; fp16 datapath (fp8 DoubleRow measured slower on
HW and too imprecise); exp on ACT except window-edge tiles which use a DVE
Schraudolph int16 bit-trick with the mask fused into the bias tensor (int16
saturation -> fp16 -0.0); denominator via S accumulation with 4x-mode
scalar_tensor_tensor adds; masks via 4x STT muls; DMA split over SP+ACT queues.
"""
import sys

sys.path.insert(0, "/opt/trn_rl_repo")

import numpy as np

import concourse.bacc as bacc
import concourse.tile as tile
import concourse.mybir as mybir

dt = mybir.dt
Alu = mybir.AluOpType

# Problem config (hardcoded per contest contract)
B, S, H, D = 2, 3072, 12, 128
LOCAL_WINDOW = 1560
SINK = 128
N_CORES = 8
PER_CORE = (B * H) // N_CORES  # 3
QB = 512
NQC = QB // 128
NKT = S // 128
SCALE = 1.0 / float(np.sqrt(D))

DELTA_W12 = 12
T_W12 = LOCAL_WINDOW - 128 * DELTA_W12  # 24
DELTA_W13 = 13
T_W13 = LOCAL_WINDOW - 128 * DELTA_W13  # -104
W13_W = 128 + T_W13  # 24
MAX_DELTA = DELTA_W13

GROUP_W = 1024

# Schraudolph fp16 exp: i16 = round(x * A16 + B16); bitcast to fp16 ~= e^x
A16 = 1024.0 / float(np.log(2.0))
C16 = 60.0
B16 = 15.0 * 1024.0 - C16
BMASK = -1.0e9  # saturates to -32768 -> fp16 -0.0

# knobs
N_EXP_DVE_FULLS = 0  # full tiles per qb whose exp runs on DVE (plain Schraudolph)


def chunk_kind(qi, kj):
    if kj == 0:
        return "diag" if qi == 0 else "full"
    delta = qi - kj
    if delta < 0 or delta > MAX_DELTA:
        return None
    if delta == 0:
        return "diag"
    if delta == DELTA_W12:
        return "w12"
    if delta == DELTA_W13:
        return "w13"
    return "full"


def qb_tiles(qb):
    lo = max(1, NQC * qb - MAX_DELTA)
    hi = min(NQC * qb + NQC - 1, NKT - 1)
    out = []
    for kj in [0] + list(range(lo, hi + 1)):
        kinds = []
        for t in range(NQC):
            k = chunk_kind(NQC * qb + t, kj)
            if k is not None:
                kinds.append((t, k))
        if not kinds:
            continue
        t0 = kinds[0][0]
        t1 = kinds[-1][0] + 1
        assert len(kinds) == t1 - t0, (qb, kj, kinds)
        eff_w = 128 * (t1 - t0)
        if kinds[-1][1] == "w13":
            eff_w -= 128 - W13_W
        kk = [k for _, k in kinds]
        if kj == 0:
            cls = "sink"
        elif "w12" in kk or "w13" in kk:
            cls = "lo" if eff_w <= 408 else "w12full"
        elif "diag" in kk:
            cls = "diag"
        else:
            cls = "full"
        out.append(dict(kj=kj, t0=t0, t1=t1, kinds=kinds, eff_w=eff_w, cls=cls))
    return out


def plan_tiles(qb):
    tiles = qb_tiles(qb)
    sink = tiles[0]
    fulls = [t for t in tiles[1:] if t["cls"] == "full"]
    diags = sorted([t for t in tiles[1:] if t["cls"] == "diag"],
                   key=lambda t: -t["eff_w"])
    w12f = [t for t in tiles[1:] if t["cls"] == "w12full"]
    los = sorted([t for t in tiles[1:] if t["cls"] == "lo"],
                 key=lambda t: -t["eff_w"])

    sink["exp"] = "act"
    for i, t in enumerate(fulls):
        t["exp"] = "dve" if i < N_EXP_DVE_FULLS else "act"
    for t in diags:
        t["exp"] = "act"
    for t in w12f:
        t["exp"] = "dve"
    for t in los:
        t["exp"] = "dve"
    return [sink] + fulls + diags + w12f + los


def plan_groups(tiles):
    groups = []
    cur = []
    pos = 0

    def close():
        nonlocal cur, pos
        if cur:
            groups.append(cur)
        cur, pos = [], 0

    for tl in tiles:
        w = tl["eff_w"]
        assert w <= 512
        start = pos
        if (start % 512) + w > 512:
            start = ((start // 512) + 1) * 512
        if start + w > GROUP_W:
            close()
            start = 0
        cur.append((tl, start))
        pos = start + w
    close()
    return groups


def build_nc(s=S, per_core=PER_CORE):
    nqb = s // QB

    nc = bacc.Bacc("TRN2", target_bir_lowering=False, debug=False)

    rqT = nc.declare_dram_parameter("rqT", [per_core, 128, s], dt.float16, isOutput=False)
    rkT = nc.declare_dram_parameter("rkT", [per_core, 128, s], dt.float16, isOutput=False)
    vT = nc.declare_dram_parameter("vT", [per_core, 128, s], dt.float16, isOutput=False)
    maskD = nc.declare_dram_parameter("maskD", [128, 128], dt.float16, isOutput=False)
    BloE = nc.declare_dram_parameter("BloE", [128, 408], dt.float32, isOutput=False)
    Bw12E = nc.declare_dram_parameter("Bw12E", [128, 512], dt.float32, isOutput=False)
    ones = nc.declare_dram_parameter("ones", [128, 128], dt.float16, isOutput=False)
    outT = nc.declare_dram_parameter("outT", [per_core, 128, s], dt.float16, isOutput=True)

    with tile.TileContext(nc) as tc:
        with (
            tc.tile_pool(name="const", bufs=1) as cpool,
            tc.tile_pool(name="big", bufs=2) as bigpool,
            tc.tile_pool(name="probs", bufs=4) as ppool,
            tc.tile_pool(name="acc", bufs=2) as apool,
            tc.tile_pool(name="outsb", bufs=3) as opool,
            tc.tile_pool(name="ps_sc", bufs=2, space="PSUM") as ps_sc,
            tc.tile_pool(name="ps_out", bufs=2, space="PSUM") as ps_out,
            tc.tile_pool(name="ps_den", bufs=2, space="PSUM") as ps_den,
        ):
            maskD_sb = cpool.tile([128, 128], dt.float16, tag="maskD")
            BloE_sb = cpool.tile([128, 408], dt.float32, tag="BloE")
            Bw12E_sb = cpool.tile([128, 512], dt.float32, tag="Bw12E")
            ones_sb = cpool.tile([128, 128], dt.float16, tag="ones")
            nc.sync.dma_start(out=maskD_sb[:], in_=maskD[:])
            nc.sync.dma_start(out=ones_sb[:], in_=ones[:])
            nc.sync.dma_start(out=BloE_sb[:], in_=BloE[:])
            nc.sync.dma_start(out=Bw12E_sb[:], in_=Bw12E[:])

            def load(u, chunks):
                rq = bigpool.tile([128, s], dt.float16, tag="rq")
                rk = bigpool.tile([128, s], dt.float16, tag="rk")
                v = bigpool.tile([128, s], dt.float16, tag="v")
                for lo, hi in chunks:
                    nc.sync.dma_start(out=rk[:, lo:hi], in_=rkT[u][:, lo:hi])
                    nc.scalar.dma_start(out=rq[:, lo:hi], in_=rqT[u][:, lo:hi])
                    nc.sync.dma_start(out=v[:, lo:hi], in_=vT[u][:, lo:hi])
                return rq, rk, v

            def attention_qb(u, rq, rk, v, qb):
                tiles = plan_tiles(qb)
                groups = plan_groups(tiles)

                outT_ps = ps_out.tile([128, QB], dt.float32, tag="outT")
                den_ps = ps_den.tile([128, QB], dt.float32, tag="den")
                S_sb = apool.tile([128, QB], dt.float16, tag="S")

                csl_base = qb * QB
                ti = 0
                si = 0
                for gtiles in groups:
                    sc = ps_sc.tile([128, GROUP_W], dt.float32, tag="sc")
                    for tl, off in gtiles:
                        ksl = slice(tl["kj"] * 128, (tl["kj"] + 1) * 128)
                        c0 = csl_base + tl["t0"] * 128
                        nc.tensor.matmul(
                            sc[:, off:off + tl["eff_w"]],
                            rk[:, ksl], rq[:, c0:c0 + tl["eff_w"]],
                            start=True, stop=True,
                        )
                    probs = ppool.tile([128, GROUP_W], dt.float16, tag="probs")

                    j = 0
                    while j < len(gtiles):
                        tl, off = gtiles[j]
                        if tl["exp"] == "act":
                            end = off + tl["eff_w"]
                            k = j + 1
                            while k < len(gtiles) and gtiles[k][1] == end \
                                    and gtiles[k][0]["exp"] == "act":
                                end = gtiles[k][1] + gtiles[k][0]["eff_w"]
                                k += 1
                            nc.scalar.activation(
                                probs[:, off:end], sc[:, off:end],
                                mybir.ActivationFunctionType.Exp, scale=SCALE,
                            )
                            j = k
                        else:
                            w = tl["eff_w"]
                            if tl["cls"] == "lo":
                                nc.vector.scalar_tensor_tensor(
                                    probs[:, off:off + w].bitcast(dt.int16),
                                    sc[:, off:off + w],
                                    A16 * SCALE, BloE_sb[:, 408 - w:408],
                                    Alu.mult, Alu.add,
                                )
                            elif tl["cls"] == "w12full":
                                nc.vector.scalar_tensor_tensor(
                                    probs[:, off:off + w].bitcast(dt.int16),
                                    sc[:, off:off + w],
                                    A16 * SCALE, Bw12E_sb[:, 0:w],
                                    Alu.mult, Alu.add,
                                )
                            else:
                                nc.vector.tensor_scalar(
                                    probs[:, off:off + w].bitcast(dt.int16),
                                    sc[:, off:off + w],
                                    A16 * SCALE, B16, Alu.mult, Alu.add,
                                )
                            j += 1

                    # diag masks via 4x STT muls on DVE
                    for tl, off in gtiles:
                        if tl["exp"] != "act":
                            continue
                        for t, kind in tl["kinds"]:
                            if kind == "full":
                                continue
                            o = off + 128 * (t - tl["t0"])
                            assert kind == "diag", (qb, tl["kj"], kind)
                            nc.vector.scalar_tensor_tensor(
                                probs[:, o:o + 128], probs[:, o:o + 128],
                                1.0, maskD_sb[:], Alu.mult, Alu.mult,
                            )

                    # S accumulation (4x STT adds) + PV
                    for tl, off in gtiles:
                        w = tl["eff_w"]
                        psl = slice(off, off + w)
                        osl = slice(tl["t0"] * 128, tl["t0"] * 128 + w)
                        ksl = slice(tl["kj"] * 128, (tl["kj"] + 1) * 128)
                        if si == 0:
                            nc.vector.tensor_copy(S_sb[:, osl], probs[:, psl])
                        else:
                            nc.vector.scalar_tensor_tensor(
                                S_sb[:, osl], probs[:, psl],
                                1.0, S_sb[:, osl], Alu.mult, Alu.add,
                            )
                        si += 1
                        nc.tensor.matmul(
                            outT_ps[:, osl], v[:, ksl], probs[:, psl],
                            start=(ti == 0), stop=(ti == len(tiles) - 1),
                        )
                        ti += 1

                nc.tensor.matmul(den_ps[:], ones_sb[:], S_sb[:], start=True, stop=True)
                rden = opool.tile([128, QB], dt.float32, tag="rden")
                nc.vector.reciprocal_approx_fast(rden[:], den_ps[:])
                outN = opool.tile([128, QB], dt.float16, tag="outN")
                nc.vector.tensor_mul(outN[:], outT_ps[:], rden[:])
                eng = nc.sync if qb % 2 == 0 else nc.scalar
                eng.dma_start(out=outT[u][:, qb * QB:(qb + 1) * QB], in_=outN[:])

            cur = load(0, [(0, 512), (512, 1536), (1536, 3072)])
            for u in range(per_core):
                nxt = None
                for qb in range(nqb):
                    attention_qb(u, cur[0], cur[1], cur[2], qb)
                    if qb == 0 and u + 1 < per_core:
                        nxt = load(u + 1, [(0, 1536), (1536, 3072)])
                cur = nxt

    nc.compile()
    return nc


def host_prep(q, k, v, cos, sin, s=S):
    """Rotary + per-core layouts on host. Returns (in_maps, units)."""
    b, _, h, d = q.shape

    cos_t = cos.astype(np.float32)
    sin_t = sin.astype(np.float32)

    def rot(x):
        x1 = x[..., 0::2]
        x2 = x[..., 1::2]
        c = cos_t[None, :, None, :]
        sn = sin_t[None, :, None, :]
        o = np.empty_like(x)
        o[..., 0::2] = x1 * c - x2 * sn
        o[..., 1::2] = x2 * c + x1 * sn
        return o

    rq = rot(q.astype(np.float32)).astype(np.float16)
    rk = rot(k.astype(np.float32)).astype(np.float16)
    v16 = v.astype(np.float16)

    p = np.arange(128)[:, None]
    c = np.arange(128)[None, :]
    maskD = (c >= p).astype(np.float16)
    ones = np.ones((128, 128), dtype=np.float16)

    bW12 = np.where((c - p) < T_W12, B16, BMASK).astype(np.float32)
    bW13 = np.where((c[:, :W13_W] - p) < T_W13, B16, BMASK).astype(np.float32)
    bF = np.full((128, 128), B16, np.float32)
    BloE = np.concatenate([bF, bF, bW12, bW13], axis=1)
    Bw12E = np.concatenate([bF, bF, bF, bW12], axis=1)

    units = [(bi, hi) for bi in range(b) for hi in range(h)]
    per = len(units) // N_CORES
    in_maps = []
    for core in range(N_CORES):
        us = units[core * per:(core + 1) * per]
        rqT = np.ascontiguousarray(np.stack([rq[bi, :, hi, :].T for bi, hi in us]))
        rkT = np.ascontiguousarray(np.stack([rk[bi, :, hi, :].T for bi, hi in us]))
        vT = np.ascontiguousarray(
            np.stack([
                v16[bi, :, hi, :].reshape(NKT, 128, 128).transpose(1, 0, 2)
                .reshape(128, s)
                for bi, hi in us
            ])
        )
        in_maps.append({
            "rqT": rqT, "rkT": rkT, "vT": vT,
            "maskD": maskD, "BloE": BloE, "Bw12E": Bw12E, "ones": ones,
        })
    return in_maps, units


_NC_CACHE = {}


def kernel(q, k, v, cos, sin):
    from concourse.bass_utils import run_bass_kernel_spmd

    q = np.asarray(q, dtype=np.float32)
    k = np.asarray(k, dtype=np.float32)
    v = np.asarray(v, dtype=np.float32)
    cos = np.asarray(cos, dtype=np.float32)
    sin = np.asarray(sin, dtype=np.float32)

    if "nc" not in _NC_CACHE:
        _NC_CACHE["nc"] = build_nc()
    nc = _NC_CACHE["nc"]

    in_maps, units = host_prep(q, k, v, cos, sin)
    res = run_bass_kernel_spmd(nc, in_maps, core_ids=list(range(N_CORES)))

    b, s, h, d = q.shape
    full = np.empty((b, s, h, d), dtype=np.float32)
    per = len(units) // N_CORES
    for core in range(N_CORES):
        o = res.results[core]["outT"]
        for i, (bi, hi) in enumerate(units[core * per:(core + 1) * per]):
            full[bi, :, hi, :] = o[i].T.astype(np.float32)
    return full
